# revision 11
# baseline (speedup 1.0000x reference)
"""Trainium2 Bass kernel for nn_CC_Decoder (hypernetwork-decoded per-pixel MLP).

Strategy (8 NeuronCores, data-parallel over batch: one sample per core):

Reference computation per sample:
  W_raw = conv1x1(x)                         # [1028, 256] channel matmul
  Wf    = W_raw @ wfine^T + wfine_b          # [1028, 256]
  layer j weights wj = Wf[257j : 257j+256], bias bj = Wf[257j+256]
  out = PE(coords)  -> 4 x (out @ wj + bj -> PReLU) -> last1 -> SiLU

Key algebraic optimization: the positional-encoding input x2 is an outer
sum over (y, x): x2[(y,x), :] = [u(y)(128) | v(x)(128)] with u = v = T
columns (T[f,t] = cos/sin(c_f * seq[t]) host table). So layer 0's K=256
contraction splits into two K=128 matmuls whose moving operands are tiny
host tables (Ty = y-columns of T broadcast along x, Trep = T tiled 4x)
instead of the 16 MB x2 tensor — layer 0 costs half the PE work of the
other layers and needs only a 4 MB Ty input streamed per tile.

Everything is kept feature-major: activations [256 feats -> 2x128
partitions, pixels free], so each layer is psum[c,px] += wj[k,c].T @
act[k,px], and per-layer PReLU+bias is a single ACT instruction per chunk
(bias rides the activation bias port; some chunks are offloaded to DVE via
prelu(h) = max(h, a*h) for engine balance). The last1 (256->3) matmuls
stack 4 pixel-tiles into one PSUM bank at 32-aligned partition offsets via
tile_position col-groups, amortizing SiLU to one instruction per 2048 px.

All matmul operands are bf16 (fp32 PSUM accumulation); measured end-to-end
relative error vs the fp32 reference is ~1e-3.
"""
import numpy as np
import ml_dtypes

bf16 = ml_dtypes.bfloat16

IMG = 128
NPX = IMG * IMG          # 16384 pixels
NF = 256                 # feature width
C1 = 1024                # conv in-channels
WD = 1028                # conv out-channels (= 4*257)
L = 4                    # generated layers
C2 = 3                   # output channels
TP = 512                 # pixel tile
NT = NPX // TP           # 32 tiles
M_ = 64
SIGMA = 10.0

_last_results = None     # stash for test.py introspection


def _host_tables():
    v0, v1 = -0.99999, 1.0
    r = (v1 - v0) / (2 * IMG)
    seq = v0 + r + 2 * r * np.arange(IMG, dtype=np.float64)
    j = np.arange(M_, dtype=np.float64)
    coeffs = 2.0 * np.pi * (SIGMA ** (j / M_))
    vp = coeffs[:, None] * seq[None, :]          # [64, 128]
    T = np.concatenate([np.cos(vp), np.sin(vp)], axis=0)  # [128, 128]
    return T.astype(np.float32)


def _build_program(alpha: float):
    import concourse.bass as bass
    import concourse.mybir as mybir
    import concourse.tile as tile
    import bir_patch_embedded  # installed below via sys.modules
    bir_patch_embedded.install()

    fp = mybir.dt.float32
    bf = mybir.dt.bfloat16
    PRELU = mybir.ActivationFunctionType.Prelu
    SILU = mybir.ActivationFunctionType.Silu
    ADD = mybir.AluOpType.add
    MULT = mybir.AluOpType.mult
    MAX = mybir.AluOpType.max

    # PReLU-on-DVE (max(h, a*h)) requires 0<=a<=1; otherwise keep all on ACT
    dve_ok = 0.0 <= alpha <= 1.0

    nc = bass.Bass()
    xb_d = nc.declare_dram_parameter("xb", [128, 8, NF], bf, isOutput=False)
    cwT_d = nc.declare_dram_parameter("cwT", [128, 8, WD], bf, isOutput=False)
    cb_d = nc.declare_dram_parameter("cb", [1, WD], bf, isOutput=False)
    wfT_d = nc.declare_dram_parameter("wfT", [128, 2, NF], bf, isOutput=False)
    wfb_d = nc.declare_dram_parameter("wfb", [1, NF], bf, isOutput=False)
    lwT_d = nc.declare_dram_parameter("lwT", [128, 2, C2], bf, isOutput=False)
    lbrep_d = nc.declare_dram_parameter("lbrep", [128, 1], fp, isOutput=False)
    Trep_d = nc.declare_dram_parameter("Trep", [128, 512], bf, isOutput=False)
    Ty_d = nc.declare_dram_parameter("Ty", [128, NT, 512], bf, isOutput=False)
    out_d = nc.declare_dram_parameter("out", [C2, NPX], fp, isOutput=True)
    out_r = out_d.rearrange("c (t x) -> c t x", x=TP)

    with tile.TileContext(nc) as tc:
        with (
            tc.tile_pool(name="wpool", bufs=1) as wp,
            tc.tile_pool(name="actp", bufs=3) as ap,
            tc.tile_pool(name="dvet", bufs=4) as dp,
            tc.tile_pool(name="outp", bufs=2) as op,
            tc.tile_pool(name="psmain", bufs=3, space="PSUM") as psm,
            tc.tile_pool(name="pslast", bufs=2, space="PSUM") as psl,
        ):
            # ---- persistent weights / tables ----
            xb = wp.tile([128, 8, NF], bf)
            cwT = wp.tile([128, 8, WD], bf)
            cb = wp.tile([1, WD], bf)
            wfT = wp.tile([128, 2, NF], bf)
            wfb = wp.tile([1, NF], bf)
            lwT = wp.tile([128, 2, C2], bf)
            lbrep = wp.tile([128, 1], fp)
            Trep = wp.tile([128, 512], bf)
            ones = wp.tile([1, 128], bf)
            Wt = wp.tile([128, 2, WD], bf)           # conv out, transposed (W^T)
            wj = [wp.tile([128, 2, NF], bf, tag=f"wj{j}", name=f"wj{j}") for j in range(L)]
            bjT = [wp.tile([128, 2], fp, tag=f"bj{j}", name=f"bj{j}") for j in range(L)]

            nc.sync.dma_start(xb[:], xb_d[:])
            for q in range(8):
                nc.sync.dma_start(cwT[:, q, :], cwT_d[:, q, :])
            nc.sync.dma_start(cb[:], cb_d[:])
            nc.sync.dma_start(wfT[:], wfT_d[:])
            nc.sync.dma_start(wfb[:], wfb_d[:])
            nc.sync.dma_start(lwT[:], lwT_d[:])
            nc.sync.dma_start(lbrep[:], lbrep_d[:])
            nc.sync.dma_start(Trep[:], Trep_d[:])
            nc.vector.memset(ones[:], 1.0)

            # ---- phase A: conv (1x1) -> W^T [hw=256 on 2 chunks, 1028 free] ----
            if True:
                psp = psm
                for m in range(2):
                    for off, sz in ((0, 512), (512, 512), (1024, 4)):
                        ps = psp.tile([128, 512], fp, tag="psmm", name="psA")
                        for q in range(8):
                            nc.tensor.matmul(
                                ps[:, :sz], xb[:, q, 128 * m:128 * (m + 1)],
                                cwT[:, q, off:off + sz],
                                start=(q == 0), stop=False)
                        nc.tensor.matmul(
                            ps[:, :sz], ones[:, 0:128], cb[:, off:off + sz],
                            start=False, stop=True)
                        nc.vector.tensor_copy(Wt[:, m, off:off + sz], ps[:, :sz])

                # ---- phase B: Wf rows -> per-layer weights + transposed biases ----
                for j in range(L):
                    r0 = 257 * j
                    for m in range(2):
                        ps = psp.tile([128, 512], fp, tag="psmm", name="psB")[:, :NF]
                        for k in range(2):
                            nc.tensor.matmul(
                                ps[:], Wt[:, k, r0 + 128 * m:r0 + 128 * (m + 1)],
                                wfT[:, k, :], start=(k == 0), stop=False)
                        nc.tensor.matmul(ps[:], ones[:, 0:128], wfb[:],
                                         start=False, stop=True)
                        nc.vector.tensor_copy(wj[j][:, m, :], ps[:])
                    for c in range(2):
                        psb = psp.tile([128, 512], fp, tag="psmm", name="psBb")[:, :1]
                        for k in range(2):
                            nc.tensor.matmul(
                                psb[:], wfT[:, k, 128 * c:128 * (c + 1)],
                                Wt[:, k, r0 + 256:r0 + 257],
                                start=(k == 0), stop=False)
                        nc.tensor.matmul(psb[:], wfb[:, 128 * c:128 * (c + 1)],
                                         ones[:, 0:1], start=False, stop=True)
                        nc.vector.tensor_copy(bjT[j][:, c:c + 1], psb[:])


            # prelu chunk-pair engine schedule per (layer, chunk):
            # 'A' -> single ACT Prelu(bias) op; 'D' -> DVE add-bias +
            # GPSIMD scale + DVE max (prelu(h) = max(h, a*h), needs 0<=a<=1)
            if dve_ok:
                assign = {(0, 1): 'D', (1, 1): 'D'}
            else:
                assign = {}

            # ---- main loop over pixel-tile pairs (weight-stationary) ----
            accL = None
            for p in range(NT // 2):
                Ty_sb = ap.tile([128, 2, TP], bf, tag="tysb", name=f"ty{p}")
                nc.sync.dma_start(Ty_sb[:], Ty_d[:, 2 * p:2 * p + 2, :])
                prev = None
                for j in range(L):
                    actj = ap.tile([128, 2, 2 * TP], bf, tag=f"act{j}",
                                   name=f"act{j}_{p}")
                    for c in range(2):
                        ps = psm.tile([128, 2, TP], fp, tag="psmm",
                                      name=f"ps{j}{c}_{p}")
                        for k in range(2):
                            for s_ in range(2):
                                t = 2 * p + s_
                                if j == 0:
                                    rhs = Ty_sb[:, s_, :] if k == 0 else Trep[:]
                                else:
                                    rhs = prev[:, k, TP * s_:TP * (s_ + 1)]
                                nc.tensor.matmul(
                                    ps[:, s_, :],
                                    wj[j][:, k, 128 * c:128 * (c + 1)], rhs,
                                    start=(k == 0), stop=(k == 1))
                        psf = ps.rearrange("p a b -> p (a b)")
                        if assign.get((j, c), 'A') == 'D':
                            h1 = dp.tile([128, 2 * TP], bf, tag="dveh",
                                         name=f"h{j}{c}_{p}")
                            t1 = dp.tile([128, 2 * TP], bf, tag="dvet",
                                         name=f"t{j}{c}_{p}")
                            nc.vector.tensor_scalar(
                                t1[:], psf, bjT[j][:, c:c + 1], alpha, ADD, MULT)
                            nc.vector.tensor_scalar(
                                h1[:], psf, bjT[j][:, c:c + 1], None, ADD)
                            nc.vector.tensor_tensor(
                                actj[:, c, :], h1[:], t1[:], MAX)
                        else:
                            nc.scalar.activation(
                                actj[:, c, :], psf, PRELU,
                                bias=bjT[j][:, c:c + 1], alpha=alpha)
                    if j == 0:
                        # Ty prefetch for the NEXT pair rides here (dep-free)
                        pass
                    prev = actj
                # last1: 4 px tiles stacked into one PSUM bank via col groups
                for s_ in range(2):
                    t = 2 * p + s_
                    g = t % 4
                    if g == 0:
                        accL = psl.tile([128, TP], fp, tag="pslastb",
                                        name=f"accL{t}")
                    for k in range(2):
                        nc.tensor.matmul(
                            accL[32 * g:32 * g + C2, :], lwT[:, k, :],
                            prev[:, k, TP * s_:TP * (s_ + 1)],
                            start=(k == 0), stop=(k == 1),
                            tile_position=(0, 32 * g))
                    if g == 3:
                        tb = t - 3
                        souf = op.tile([128, TP], fp, tag="souf",
                                       name=f"souf{t}")
                        nc.scalar.activation(souf[0:99, :], accL[0:99, :],
                                             SILU, bias=lbrep[0:99, 0:1])
                        for c in range(C2):
                            nc.sync.dma_start(out_r[c, tb:tb + 4, :],
                                              souf[c:c + 97:32, :])
    return nc


def kernel(x, conv_w, conv_b, wfine_w, wfine_b, last1_w, last1_b, prelu_a,
           **_ignored):
    global _last_results
    from concourse.bass_utils import run_bass_kernel_spmd

    x = np.asarray(x)
    B = x.shape[0]
    assert x.shape == (B, C1, 16, 16) and B == 8, x.shape

    conv_w = np.asarray(conv_w, np.float32)      # [1028, 1024]
    conv_b = np.asarray(conv_b, np.float32)      # [1028]
    wfine_w = np.asarray(wfine_w, np.float32)    # [256, 256]
    wfine_b = np.asarray(wfine_b, np.float32)    # [256]
    last1_w = np.asarray(last1_w, np.float32)    # [3, 256]
    last1_b = np.asarray(last1_b, np.float32)    # [3]
    alpha = float(np.asarray(prelu_a).reshape(-1)[0])

    # host-side shared operands (bf16)
    cwT = np.ascontiguousarray(
        conv_w.T.reshape(8, 128, WD).transpose(1, 0, 2)).astype(bf16)
    cb = conv_b.reshape(1, WD).astype(bf16)
    wfT = np.ascontiguousarray(
        wfine_w.T.reshape(2, 128, NF).transpose(1, 0, 2)).astype(bf16)
    wfb = wfine_b.reshape(1, NF).astype(bf16)
    lwT = np.ascontiguousarray(
        last1_w.T.reshape(2, 128, C2).transpose(1, 0, 2)).astype(bf16)
    lbrep = np.zeros((128, 1), np.float32)
    for g in range(4):
        lbrep[32 * g:32 * g + C2, 0] = last1_b
    Tt = _host_tables()
    Trep = np.tile(Tt, (1, 4)).astype(bf16)
    Ty = np.ascontiguousarray(
        np.broadcast_to(Tt[:, :, None], (128, 128, 128)).reshape(128, NT, 512)
    ).astype(bf16)

    nc = _build_program(alpha)

    in_maps = []
    for b in range(B):
        xb = np.ascontiguousarray(
            x[b].reshape(8, 128, NF).transpose(1, 0, 2)).astype(bf16)
        in_maps.append({"xb": xb, "cwT": cwT, "cb": cb, "wfT": wfT,
                        "wfb": wfb, "lwT": lwT, "lbrep": lbrep,
                        "Trep": Trep, "Ty": Ty})

    res = run_bass_kernel_spmd(nc, in_maps, list(range(8)))
    _last_results = res
    out = np.stack([res.results[b]["out"].reshape(C2, IMG, IMG)
                    for b in range(B)])
    return out.astype(np.float32)


# ---------------------------------------------------------------------------
# Embedded walrus workaround (kernel.py must be self-contained): this walrus
# build accepts at most ONE sync wait per instruction; Tile attaches several.
# Split them into preceding single-wait NoOps at the BIR-JSON level, and make
# the TileContext tail drain emit one single-wait drain per logical proc.
# ---------------------------------------------------------------------------
import sys as _sys
import types as _types

_patch_mod = _types.ModuleType("bir_patch_embedded")
_patch_src = r'''
import json

def install():
    import concourse.bass_utils as _bu
    import concourse.bass2jax as _b2j
    import concourse.tile as _tile
    from concourse.vector_clock import ScopedClock, VectorClock

    if getattr(_bu, "_wait_legalizer_installed", False):
        return
    _bu._wait_legalizer_installed = True
    _orig_compile = _bu.compile_bir_kernel

    def _merge_ldweights(m):
        """Re-merge tile-legalize's split Ldweights into self-loading
        Matmults so walrus codegen can apply FWL / ldw dedupe."""
        for fn in m.get("functions", []):
            for bb in fn.get("blocks", []):
                instrs = bb.get("instructions", [])
                out = []
                i = 0
                while i < len(instrs):
                    ins = instrs[i]
                    if ins.get("opcode") == "Ldweights":
                        wap = json.dumps(ins["ins"][0], sort_keys=True)
                        # find the next Matmult on this engine using these
                        # weights (stationary operand = ins[1])
                        tgt = None
                        for k in range(i + 1, min(i + 8, len(instrs))):
                            nxt = instrs[k]
                            if nxt.get("engine") != ins.get("engine"):
                                continue
                            if nxt.get("opcode") == "Matmult" and json.dumps(
                                    nxt["ins"][1], sort_keys=True) == wap:
                                tgt = nxt
                            break
                        if tgt is not None:
                            tgt["ldweights"] = True
                            si, ti = ins.get("sync_info") or {}, tgt.setdefault(
                                "sync_info", {"on_wait": [], "on_update": []})
                            ti.setdefault("on_wait", []).extend(
                                si.get("on_wait") or [])
                            ti.setdefault("on_update", []).extend(
                                si.get("on_update") or [])
                            i += 1
                            continue
                    out.append(ins)
                    i += 1
                bb["instructions"] = out
        return m

    def _legalize_waits(bir_json):
        m = json.loads(bir_json)
        m = _merge_ldweights(m)
        cnt = 0
        changed = True
        for fn in m.get("functions", []):
            for bb in fn.get("blocks", []):
                new_instrs = []
                for ins in bb.get("instructions", []):
                    si = ins.get("sync_info")
                    ow = (si or {}).get("on_wait") or []
                    if len(ow) > 1:
                        changed = True
                        for w in ow[:-1]:
                            cnt += 1
                            new_instrs.append({
                                "engine": ins["engine"],
                                "ins": [], "outs": [],
                                "name": "WSPLIT-%d" % cnt,
                                "opcode": "NoOp",
                                "sync_info": {"on_update": [], "on_wait": [w]},
                                "debug": ins.get("debug", 0),
                            })
                        si["on_wait"] = [ow[-1]]
                    new_instrs.append(ins)
                bb["instructions"] = new_instrs
        if not changed:
            return bir_json
        return json.dumps(m).encode()

    def _compile_legalized(bir_json, tmpdir, neff_name="file.neff"):
        return _orig_compile(_legalize_waits(bir_json), tmpdir, neff_name)

    _bu.compile_bir_kernel = _compile_legalized
    _b2j.compile_bir_kernel = _compile_legalized

    import os
    if os.environ.get("BASS_LDW_OPT", "1") != "0":
        _orig_verify = _bu.bir_verify_and_optimise

        def _verify_ldwopt(tmpdir, inp="bir.json", outp="file.neff", arch=None,
                           *, dve_root=None):
            saved = _bu.run_command

            def run_cmd(cmd, **kw):
                cmd = [c.replace("--enable-ldw-opt=false",
                                 "--enable-ldw-opt=true")
                       if isinstance(c, str) else c for c in cmd]
                return saved(cmd, **kw)
            _bu.run_command = run_cmd
            try:
                return _orig_verify(tmpdir, inp, outp, arch, dve_root=dve_root)
            finally:
                _bu.run_command = saved
        _bu.bir_verify_and_optimise = _verify_ldwopt

    def _drain_and_barrier_split(self, tick_clock, wait_clock):
        nc = self.nc
        vclock = tick_clock.global_clock
        n = len(vclock)
        for p in range(n):
            t = vclock[p]
            if t <= 0:
                continue
            v = VectorClock([0] * n)
            v.require_at_least(p, t)
            d = nc.sync.drain()
            wait_clock.add_sem_waits(d.ins, ScopedClock({None: v}))
        nc.all_engine_barrier()
        popped = nc._tile_sem_poison_stack.pop()
        assert popped is self._sem_poison
        nc.clear_and_free_semaphores(list(self.sems.allocated().values()))
        nc.all_engine_barrier()

    _tile.TileContext._drain_and_barrier = _drain_and_barrier_split
'''
exec(_patch_src, _patch_mod.__dict__)
_sys.modules["bir_patch_embedded"] = _patch_mod


# revision 12
# speedup vs baseline: 1.2826x; 1.2826x over previous
"""Trainium2 Bass kernel for nn_CC_Decoder (hypernetwork-decoded per-pixel MLP).

Strategy (8 NeuronCores, data-parallel over batch: one sample per core):

Reference computation per sample:
  W_raw = conv1x1(x)                         # [1028, 256] channel matmul
  Wf    = W_raw @ wfine^T + wfine_b          # [1028, 256]
  layer j weights wj = Wf[257j : 257j+256], bias bj = Wf[257j+256]
  out = PE(coords)  -> 4 x (out @ wj + bj -> PReLU) -> last1 -> SiLU

Key algebraic optimization: the positional-encoding input x2 is an outer
sum over (y, x): x2[(y,x), :] = [u(y)(128) | v(x)(128)] with u = v = T
columns (T[f,t] = cos/sin(c_f * seq[t]) host table). So layer 0's K=256
contraction splits into two K=128 matmuls whose moving operands are tiny
host tables (Ty = y-columns of T broadcast along x, Trep = T tiled 4x)
instead of the 16 MB x2 tensor — layer 0 costs half the PE work of the
other layers and needs only a 4 MB Ty input streamed per tile.

Everything is kept feature-major: activations [256 feats -> 2x128
partitions, pixels free], so each layer is psum[c,px] += wj[k,c].T @
act[k,px], and per-layer PReLU+bias is a single ACT instruction per chunk
(bias rides the activation bias port; some chunks are offloaded to DVE via
prelu(h) = max(h, a*h) for engine balance). The last1 (256->3) matmuls
stack 4 pixel-tiles into one PSUM bank at 32-aligned partition offsets via
tile_position col-groups, amortizing SiLU to one instruction per 2048 px.

All matmul operands are bf16 (fp32 PSUM accumulation); measured end-to-end
relative error vs the fp32 reference is ~1e-3.
"""
import numpy as np
import ml_dtypes

bf16 = ml_dtypes.bfloat16

IMG = 128
NPX = IMG * IMG          # 16384 pixels
NF = 256                 # feature width
C1 = 1024                # conv in-channels
WD = 1028                # conv out-channels (= 4*257)
L = 4                    # generated layers
C2 = 3                   # output channels
TP = 512                 # pixel tile
NT = NPX // TP           # 32 tiles
M_ = 64
SIGMA = 10.0

_last_results = None     # stash for test.py introspection


def _host_tables():
    v0, v1 = -0.99999, 1.0
    r = (v1 - v0) / (2 * IMG)
    seq = v0 + r + 2 * r * np.arange(IMG, dtype=np.float64)
    j = np.arange(M_, dtype=np.float64)
    coeffs = 2.0 * np.pi * (SIGMA ** (j / M_))
    vp = coeffs[:, None] * seq[None, :]          # [64, 128]
    T = np.concatenate([np.cos(vp), np.sin(vp)], axis=0)  # [128, 128]
    return T.astype(np.float32)


def _build_program(alpha: float):
    import concourse.bass as bass
    import concourse.mybir as mybir
    import concourse.tile as tile
    import bir_patch_embedded  # installed below via sys.modules
    bir_patch_embedded.install()

    fp = mybir.dt.float32
    bf = mybir.dt.bfloat16
    PRELU = mybir.ActivationFunctionType.Prelu
    SILU = mybir.ActivationFunctionType.Silu
    ADD = mybir.AluOpType.add
    MULT = mybir.AluOpType.mult
    MAX = mybir.AluOpType.max

    # PReLU-on-DVE (max(h, a*h)) requires 0<=a<=1; otherwise keep all on ACT
    dve_ok = 0.0 <= alpha <= 1.0

    nc = bass.Bass()
    xb_d = nc.declare_dram_parameter("xb", [128, 8, NF], bf, isOutput=False)
    cwT_d = nc.declare_dram_parameter("cwT", [128, 8, WD], bf, isOutput=False)
    cb_d = nc.declare_dram_parameter("cb", [1, WD], bf, isOutput=False)
    wfT_d = nc.declare_dram_parameter("wfT", [128, 2, NF], bf, isOutput=False)
    wfb_d = nc.declare_dram_parameter("wfb", [1, NF], bf, isOutput=False)
    lwT_d = nc.declare_dram_parameter("lwT", [128, 2, C2], bf, isOutput=False)
    lbrep_d = nc.declare_dram_parameter("lbrep", [128, 1], fp, isOutput=False)
    Trep_d = nc.declare_dram_parameter("Trep", [128, 512], bf, isOutput=False)
    Ty_d = nc.declare_dram_parameter("Ty", [128, NT, 512], bf, isOutput=False)
    out_d = nc.declare_dram_parameter("out", [C2, NPX], fp, isOutput=True)
    out_r = out_d.rearrange("c (t x) -> c t x", x=TP)

    with tile.TileContext(nc) as tc:
        with (
            tc.tile_pool(name="wpool", bufs=1) as wp,
            tc.tile_pool(name="actp", bufs=3) as ap,
            tc.tile_pool(name="dvet", bufs=4) as dp,
            tc.tile_pool(name="outp", bufs=2) as op,
            tc.tile_pool(name="psmain", bufs=3, space="PSUM") as psm,
            tc.tile_pool(name="pslast", bufs=2, space="PSUM") as psl,
        ):
            # ---- persistent weights / tables ----
            xb = wp.tile([128, 8, NF], bf)
            cwT = wp.tile([128, 8, WD], bf)
            cb = wp.tile([1, WD], bf)
            wfT = wp.tile([128, 2, NF], bf)
            wfb = wp.tile([1, NF], bf)
            lwT = wp.tile([128, 2, C2], bf)
            lbrep = wp.tile([128, 1], fp)
            Trep = wp.tile([128, 512], bf)
            ones = wp.tile([1, 128], bf)
            Wt = wp.tile([128, 2, WD], bf)           # conv out, transposed (W^T)
            wj = [wp.tile([128, 2, NF], bf, tag=f"wj{j}", name=f"wj{j}") for j in range(L)]
            bjT = [wp.tile([128, 2], fp, tag=f"bj{j}", name=f"bj{j}") for j in range(L)]

            nc.sync.dma_start(xb[:], xb_d[:])
            for q in range(8):
                nc.sync.dma_start(cwT[:, q, :], cwT_d[:, q, :])
            nc.sync.dma_start(cb[:], cb_d[:])
            nc.sync.dma_start(wfT[:], wfT_d[:])
            nc.sync.dma_start(wfb[:], wfb_d[:])
            nc.sync.dma_start(lwT[:], lwT_d[:])
            nc.sync.dma_start(lbrep[:], lbrep_d[:])
            nc.sync.dma_start(Trep[:], Trep_d[:])
            nc.vector.memset(ones[:], 1.0)

            # ---- phase A: conv (1x1) -> W^T [hw=256 on 2 chunks, 1028 free] ----
            if True:
                psp = psm
                for m in range(2):
                    for off, sz in ((0, 512), (512, 512), (1024, 4)):
                        ps = psp.tile([128, 512], fp, tag="psmm", name="psA")
                        for q in range(8):
                            nc.tensor.matmul(
                                ps[:, :sz], xb[:, q, 128 * m:128 * (m + 1)],
                                cwT[:, q, off:off + sz],
                                start=(q == 0), stop=False)
                        nc.tensor.matmul(
                            ps[:, :sz], ones[:, 0:128], cb[:, off:off + sz],
                            start=False, stop=True)
                        nc.vector.tensor_copy(Wt[:, m, off:off + sz], ps[:, :sz])

                # ---- phase B: Wf rows -> per-layer weights + transposed biases ----
                for j in range(L):
                    r0 = 257 * j
                    for m in range(2):
                        ps = psp.tile([128, 512], fp, tag="psmm", name="psB")[:, :NF]
                        for k in range(2):
                            nc.tensor.matmul(
                                ps[:], Wt[:, k, r0 + 128 * m:r0 + 128 * (m + 1)],
                                wfT[:, k, :], start=(k == 0), stop=False)
                        nc.tensor.matmul(ps[:], ones[:, 0:128], wfb[:],
                                         start=False, stop=True)
                        nc.vector.tensor_copy(wj[j][:, m, :], ps[:])
                    for c in range(2):
                        psb = psp.tile([128, 512], fp, tag="psmm", name="psBb")[:, :1]
                        for k in range(2):
                            nc.tensor.matmul(
                                psb[:], wfT[:, k, 128 * c:128 * (c + 1)],
                                Wt[:, k, r0 + 256:r0 + 257],
                                start=(k == 0), stop=False)
                        nc.tensor.matmul(psb[:], wfb[:, 128 * c:128 * (c + 1)],
                                         ones[:, 0:1], start=False, stop=True)
                        nc.vector.tensor_copy(bjT[j][:, c:c + 1], psb[:])


            # prelu chunk-pair engine schedule per (layer, chunk):
            # 'A' -> single ACT Prelu(bias) op; 'D' -> DVE add-bias +
            # GPSIMD scale + DVE max (prelu(h) = max(h, a*h), needs 0<=a<=1)
            if dve_ok:
                assign = {(0, 1): 'D', (2, 1): 'D'}
            else:
                assign = {}

            # ---- main loop over pixel-tile pairs (weight-stationary) ----
            accL = None
            for p in range(NT // 2):
                Ty_sb = ap.tile([128, 2, TP], bf, tag="tysb", name=f"ty{p}")
                nc.sync.dma_start(Ty_sb[:], Ty_d[:, 2 * p:2 * p + 2, :])
                prev = None
                for j in range(L):
                    actj = ap.tile([128, 2, 2 * TP], bf, tag=f"act{j}",
                                   name=f"act{j}_{p}")
                    for c in range(2):
                        ps = psm.tile([128, 2, TP], fp, tag="psmm",
                                      name=f"ps{j}{c}_{p}")
                        for k in range(2):
                            for s_ in range(2):
                                t = 2 * p + s_
                                if j == 0:
                                    rhs = Ty_sb[:, s_, :] if k == 0 else Trep[:]
                                else:
                                    rhs = prev[:, k, TP * s_:TP * (s_ + 1)]
                                nc.tensor.matmul(
                                    ps[:, s_, :],
                                    wj[j][:, k, 128 * c:128 * (c + 1)], rhs,
                                    start=(k == 0), stop=(k == 1))
                        psf = ps.rearrange("p a b -> p (a b)")
                        if assign.get((j, c), 'A') == 'D':
                            h1 = dp.tile([128, 2 * TP], bf, tag="dveh",
                                         name=f"h{j}{c}_{p}")
                            t1 = dp.tile([128, 2 * TP], bf, tag="dvet",
                                         name=f"t{j}{c}_{p}")
                            nc.vector.tensor_scalar(
                                h1[:], psf, bjT[j][:, c:c + 1], None, ADD)
                            nc.vector.tensor_scalar(
                                t1[:], h1[:], alpha, None, MULT)
                            nc.vector.tensor_tensor(
                                actj[:, c, :], h1[:], t1[:], MAX)
                        else:
                            nc.scalar.activation(
                                actj[:, c, :], psf, PRELU,
                                bias=bjT[j][:, c:c + 1], alpha=alpha)
                    if j == 0:
                        # Ty prefetch for the NEXT pair rides here (dep-free)
                        pass
                    prev = actj
                # last1: 4 px tiles stacked into one PSUM bank via col groups
                for s_ in range(2):
                    t = 2 * p + s_
                    g = t % 4
                    if g == 0:
                        accL = psl.tile([128, TP], fp, tag="pslastb",
                                        name=f"accL{t}")
                    for k in range(2):
                        nc.tensor.matmul(
                            accL[32 * g:32 * g + C2, :], lwT[:, k, :],
                            prev[:, k, TP * s_:TP * (s_ + 1)],
                            start=(k == 0), stop=(k == 1),
                            tile_position=(0, 32 * g))
                    if g == 3:
                        tb = t - 3
                        souf = op.tile([128, TP], fp, tag="souf",
                                       name=f"souf{t}")
                        nc.scalar.activation(souf[0:99, :], accL[0:99, :],
                                             SILU, bias=lbrep[0:99, 0:1])
                        for c in range(C2):
                            nc.sync.dma_start(out_r[c, tb:tb + 4, :],
                                              souf[c:c + 97:32, :])
    return nc


def kernel(x, conv_w, conv_b, wfine_w, wfine_b, last1_w, last1_b, prelu_a,
           **_ignored):
    global _last_results
    from concourse.bass_utils import run_bass_kernel_spmd

    x = np.asarray(x)
    B = x.shape[0]
    assert x.shape == (B, C1, 16, 16) and B == 8, x.shape

    conv_w = np.asarray(conv_w, np.float32)      # [1028, 1024]
    conv_b = np.asarray(conv_b, np.float32)      # [1028]
    wfine_w = np.asarray(wfine_w, np.float32)    # [256, 256]
    wfine_b = np.asarray(wfine_b, np.float32)    # [256]
    last1_w = np.asarray(last1_w, np.float32)    # [3, 256]
    last1_b = np.asarray(last1_b, np.float32)    # [3]
    alpha = float(np.asarray(prelu_a).reshape(-1)[0])

    # host-side shared operands (bf16)
    cwT = np.ascontiguousarray(
        conv_w.T.reshape(8, 128, WD).transpose(1, 0, 2)).astype(bf16)
    cb = conv_b.reshape(1, WD).astype(bf16)
    wfT = np.ascontiguousarray(
        wfine_w.T.reshape(2, 128, NF).transpose(1, 0, 2)).astype(bf16)
    wfb = wfine_b.reshape(1, NF).astype(bf16)
    lwT = np.ascontiguousarray(
        last1_w.T.reshape(2, 128, C2).transpose(1, 0, 2)).astype(bf16)
    lbrep = np.zeros((128, 1), np.float32)
    for g in range(4):
        lbrep[32 * g:32 * g + C2, 0] = last1_b
    Tt = _host_tables()
    Trep = np.tile(Tt, (1, 4)).astype(bf16)
    Ty = np.ascontiguousarray(
        np.broadcast_to(Tt[:, :, None], (128, 128, 128)).reshape(128, NT, 512)
    ).astype(bf16)

    nc = _build_program(alpha)

    in_maps = []
    for b in range(B):
        xb = np.ascontiguousarray(
            x[b].reshape(8, 128, NF).transpose(1, 0, 2)).astype(bf16)
        in_maps.append({"xb": xb, "cwT": cwT, "cb": cb, "wfT": wfT,
                        "wfb": wfb, "lwT": lwT, "lbrep": lbrep,
                        "Trep": Trep, "Ty": Ty})

    res = run_bass_kernel_spmd(nc, in_maps, list(range(8)))
    _last_results = res
    out = np.stack([res.results[b]["out"].reshape(C2, IMG, IMG)
                    for b in range(B)])
    return out.astype(np.float32)


# ---------------------------------------------------------------------------
# Embedded walrus workaround (kernel.py must be self-contained): this walrus
# build accepts at most ONE sync wait per instruction; Tile attaches several.
# Split them into preceding single-wait NoOps at the BIR-JSON level, and make
# the TileContext tail drain emit one single-wait drain per logical proc.
# ---------------------------------------------------------------------------
import sys as _sys
import types as _types

_patch_mod = _types.ModuleType("bir_patch_embedded")
_patch_src = r'''
import json

def install():
    import concourse.bass_utils as _bu
    import concourse.bass2jax as _b2j
    import concourse.tile as _tile
    from concourse.vector_clock import ScopedClock, VectorClock

    if getattr(_bu, "_wait_legalizer_installed", False):
        return
    _bu._wait_legalizer_installed = True
    _orig_compile = _bu.compile_bir_kernel

    def _merge_ldweights(m):
        """Re-merge tile-legalize's split Ldweights into self-loading
        Matmults so walrus codegen can apply FWL / ldw dedupe."""
        for fn in m.get("functions", []):
            for bb in fn.get("blocks", []):
                instrs = bb.get("instructions", [])
                out = []
                i = 0
                while i < len(instrs):
                    ins = instrs[i]
                    if ins.get("opcode") == "Ldweights":
                        wap = json.dumps(ins["ins"][0], sort_keys=True)
                        # find the next Matmult on this engine using these
                        # weights (stationary operand = ins[1])
                        tgt = None
                        for k in range(i + 1, min(i + 8, len(instrs))):
                            nxt = instrs[k]
                            if nxt.get("engine") != ins.get("engine"):
                                continue
                            if nxt.get("opcode") == "Matmult" and json.dumps(
                                    nxt["ins"][1], sort_keys=True) == wap:
                                tgt = nxt
                            break
                        if tgt is not None:
                            tgt["ldweights"] = True
                            si, ti = ins.get("sync_info") or {}, tgt.setdefault(
                                "sync_info", {"on_wait": [], "on_update": []})
                            ti.setdefault("on_wait", []).extend(
                                si.get("on_wait") or [])
                            ti.setdefault("on_update", []).extend(
                                si.get("on_update") or [])
                            i += 1
                            continue
                    out.append(ins)
                    i += 1
                bb["instructions"] = out
        return m

    def _legalize_waits(bir_json):
        m = json.loads(bir_json)
        m = _merge_ldweights(m)
        cnt = 0
        changed = True
        for fn in m.get("functions", []):
            for bb in fn.get("blocks", []):
                new_instrs = []
                for ins in bb.get("instructions", []):
                    si = ins.get("sync_info")
                    ow = (si or {}).get("on_wait") or []
                    if len(ow) > 1:
                        changed = True
                        for w in ow[:-1]:
                            cnt += 1
                            new_instrs.append({
                                "engine": ins["engine"],
                                "ins": [], "outs": [],
                                "name": "WSPLIT-%d" % cnt,
                                "opcode": "NoOp",
                                "sync_info": {"on_update": [], "on_wait": [w]},
                                "debug": ins.get("debug", 0),
                            })
                        si["on_wait"] = [ow[-1]]
                    new_instrs.append(ins)
                bb["instructions"] = new_instrs
        if not changed:
            return bir_json
        return json.dumps(m).encode()

    def _compile_legalized(bir_json, tmpdir, neff_name="file.neff"):
        return _orig_compile(_legalize_waits(bir_json), tmpdir, neff_name)

    _bu.compile_bir_kernel = _compile_legalized
    _b2j.compile_bir_kernel = _compile_legalized

    import os
    if os.environ.get("BASS_LDW_OPT", "1") != "0":
        _orig_verify = _bu.bir_verify_and_optimise

        def _verify_ldwopt(tmpdir, inp="bir.json", outp="file.neff", arch=None,
                           *, dve_root=None):
            saved = _bu.run_command

            def run_cmd(cmd, **kw):
                cmd = [c.replace("--enable-ldw-opt=false",
                                 "--enable-ldw-opt=true")
                       if isinstance(c, str) else c for c in cmd]
                return saved(cmd, **kw)
            _bu.run_command = run_cmd
            try:
                return _orig_verify(tmpdir, inp, outp, arch, dve_root=dve_root)
            finally:
                _bu.run_command = saved
        _bu.bir_verify_and_optimise = _verify_ldwopt

    def _drain_and_barrier_split(self, tick_clock, wait_clock):
        nc = self.nc
        vclock = tick_clock.global_clock
        n = len(vclock)
        for p in range(n):
            t = vclock[p]
            if t <= 0:
                continue
            v = VectorClock([0] * n)
            v.require_at_least(p, t)
            d = nc.sync.drain()
            wait_clock.add_sem_waits(d.ins, ScopedClock({None: v}))
        nc.all_engine_barrier()
        popped = nc._tile_sem_poison_stack.pop()
        assert popped is self._sem_poison
        nc.clear_and_free_semaphores(list(self.sems.allocated().values()))
        nc.all_engine_barrier()

    _tile.TileContext._drain_and_barrier = _drain_and_barrier_split
'''
exec(_patch_src, _patch_mod.__dict__)
_sys.modules["bir_patch_embedded"] = _patch_mod


# revision 13
# speedup vs baseline: 1.3388x; 1.0438x over previous
"""Trainium2 Bass kernel for nn_CC_Decoder (hypernetwork-decoded per-pixel MLP).

Strategy (8 NeuronCores, data-parallel over batch: one sample per core):

Reference computation per sample:
  W_raw = conv1x1(x)                         # [1028, 256] channel matmul
  Wf    = W_raw @ wfine^T + wfine_b          # [1028, 256]
  layer j weights wj = Wf[257j : 257j+256], bias bj = Wf[257j+256]
  out = PE(coords)  -> 4 x (out @ wj + bj -> PReLU) -> last1 -> SiLU

Key algebraic optimization: the positional-encoding input x2 is an outer
sum over (y, x): x2[(y,x), :] = [u(y)(128) | v(x)(128)] with u = v = T
columns (T[f,t] = cos/sin(c_f * seq[t]) host table). So layer 0's K=256
contraction splits into two K=128 matmuls whose moving operands are tiny
host tables (Ty = y-columns of T broadcast along x, Trep = T tiled 4x)
instead of the 16 MB x2 tensor — layer 0 costs half the PE work of the
other layers and needs only a 4 MB Ty input streamed per tile.

Everything is kept feature-major: activations [256 feats -> 2x128
partitions, pixels free], so each layer is psum[c,px] += wj[k,c].T @
act[k,px], and per-layer PReLU+bias is a single ACT instruction per chunk
(bias rides the activation bias port; some chunks are offloaded to DVE via
prelu(h) = max(h, a*h) for engine balance). The last1 (256->3) matmuls
stack 4 pixel-tiles into one PSUM bank at 32-aligned partition offsets via
tile_position col-groups, amortizing SiLU to one instruction per 2048 px.

All matmul operands are bf16 (fp32 PSUM accumulation); measured end-to-end
relative error vs the fp32 reference is ~1e-3.
"""
import numpy as np
import ml_dtypes

bf16 = ml_dtypes.bfloat16

IMG = 128
NPX = IMG * IMG          # 16384 pixels
NF = 256                 # feature width
C1 = 1024                # conv in-channels
WD = 1028                # conv out-channels (= 4*257)
L = 4                    # generated layers
C2 = 3                   # output channels
TP = 512                 # pixel tile
NT = NPX // TP           # 32 tiles
M_ = 64
SIGMA = 10.0

_last_results = None     # stash for test.py introspection


def _host_tables():
    v0, v1 = -0.99999, 1.0
    r = (v1 - v0) / (2 * IMG)
    seq = v0 + r + 2 * r * np.arange(IMG, dtype=np.float64)
    j = np.arange(M_, dtype=np.float64)
    coeffs = 2.0 * np.pi * (SIGMA ** (j / M_))
    vp = coeffs[:, None] * seq[None, :]          # [64, 128]
    T = np.concatenate([np.cos(vp), np.sin(vp)], axis=0)  # [128, 128]
    return T.astype(np.float32)


def _build_program(alpha: float):
    import concourse.bass as bass
    import concourse.mybir as mybir
    import concourse.tile as tile
    import bir_patch_embedded  # installed below via sys.modules
    bir_patch_embedded.install()

    fp = mybir.dt.float32
    bf = mybir.dt.bfloat16
    PRELU = mybir.ActivationFunctionType.Prelu
    SILU = mybir.ActivationFunctionType.Silu
    ADD = mybir.AluOpType.add
    MULT = mybir.AluOpType.mult
    MAX = mybir.AluOpType.max

    # PReLU-on-DVE (max(h, a*h)) requires 0<=a<=1; otherwise keep all on ACT
    dve_ok = 0.0 <= alpha <= 1.0

    nc = bass.Bass()
    xb_d = nc.declare_dram_parameter("xb", [128, 8, NF], bf, isOutput=False)
    cwT_d = nc.declare_dram_parameter("cwT", [128, 8, WD], bf, isOutput=False)
    cb_d = nc.declare_dram_parameter("cb", [1, WD], bf, isOutput=False)
    wfT_d = nc.declare_dram_parameter("wfT", [128, 2, NF], bf, isOutput=False)
    wfb_d = nc.declare_dram_parameter("wfb", [1, NF], bf, isOutput=False)
    lwT_d = nc.declare_dram_parameter("lwT", [128, 2, C2], bf, isOutput=False)
    lbrep_d = nc.declare_dram_parameter("lbrep", [128, 1], fp, isOutput=False)
    Trep_d = nc.declare_dram_parameter("Trep", [128, 512], bf, isOutput=False)
    Ty_d = nc.declare_dram_parameter("Ty", [128, NT, 512], bf, isOutput=False)
    out_d = nc.declare_dram_parameter("out", [C2, NPX], fp, isOutput=True)
    out_r = out_d.rearrange("c (t x) -> c t x", x=TP)

    with tile.TileContext(nc) as tc:
        with (
            tc.tile_pool(name="wpool", bufs=1) as wp,
            tc.tile_pool(name="actp", bufs=3) as ap,
            tc.tile_pool(name="dvet", bufs=4) as dp,
            tc.tile_pool(name="outp", bufs=2) as op,
            tc.tile_pool(name="psmain", bufs=3, space="PSUM") as psm,
            tc.tile_pool(name="pslast", bufs=2, space="PSUM") as psl,
        ):
            # ---- persistent weights / tables ----
            xb = wp.tile([128, 8, NF], bf)
            cwT = wp.tile([128, 8, WD], bf)
            cb = wp.tile([1, WD], bf)
            wfT = wp.tile([128, 2, NF], bf)
            wfb = wp.tile([1, NF], bf)
            lwT = wp.tile([128, 2, C2], bf)
            lbrep = wp.tile([128, 1], fp)
            Trep = wp.tile([128, 512], bf)
            ones = wp.tile([1, 128], bf)
            Wt = wp.tile([128, 2, WD], bf)           # conv out, transposed (W^T)
            wj = [wp.tile([128, 2, NF], bf, tag=f"wj{j}", name=f"wj{j}") for j in range(L)]
            bjT = [wp.tile([128, 2], fp, tag=f"bj{j}", name=f"bj{j}") for j in range(L)]

            nc.sync.dma_start(xb[:], xb_d[:])
            for q in range(8):
                nc.sync.dma_start(cwT[:, q, :], cwT_d[:, q, :])
            nc.sync.dma_start(cb[:], cb_d[:])
            nc.sync.dma_start(wfT[:], wfT_d[:])
            nc.sync.dma_start(wfb[:], wfb_d[:])
            nc.sync.dma_start(lwT[:], lwT_d[:])
            nc.sync.dma_start(lbrep[:], lbrep_d[:])
            nc.sync.dma_start(Trep[:], Trep_d[:])
            nc.vector.memset(ones[:], 1.0)

            # ---- phase A: conv (1x1) -> W^T [hw=256 on 2 chunks, 1028 free] ----
            if True:
                psp = psm
                for m in range(2):
                    for off, sz in ((0, 512), (512, 512), (1024, 4)):
                        ps = psp.tile([128, 512], fp, tag="psmm", name="psA")
                        for q in range(8):
                            nc.tensor.matmul(
                                ps[:, :sz], xb[:, q, 128 * m:128 * (m + 1)],
                                cwT[:, q, off:off + sz],
                                start=(q == 0), stop=False)
                        nc.tensor.matmul(
                            ps[:, :sz], ones[:, 0:128], cb[:, off:off + sz],
                            start=False, stop=True)
                        nc.vector.tensor_copy(Wt[:, m, off:off + sz], ps[:, :sz])

                # ---- phase B: Wf rows -> per-layer weights + transposed biases ----
                for j in range(L):
                    r0 = 257 * j
                    for m in range(2):
                        ps = psp.tile([128, 512], fp, tag="psmm", name="psB")[:, :NF]
                        for k in range(2):
                            nc.tensor.matmul(
                                ps[:], Wt[:, k, r0 + 128 * m:r0 + 128 * (m + 1)],
                                wfT[:, k, :], start=(k == 0), stop=False)
                        nc.tensor.matmul(ps[:], ones[:, 0:128], wfb[:],
                                         start=False, stop=True)
                        nc.vector.tensor_copy(wj[j][:, m, :], ps[:])
                    for c in range(2):
                        psb = psp.tile([128, 512], fp, tag="psmm", name="psBb")[:, :1]
                        for k in range(2):
                            nc.tensor.matmul(
                                psb[:], wfT[:, k, 128 * c:128 * (c + 1)],
                                Wt[:, k, r0 + 256:r0 + 257],
                                start=(k == 0), stop=False)
                        nc.tensor.matmul(psb[:], wfb[:, 128 * c:128 * (c + 1)],
                                         ones[:, 0:1], start=False, stop=True)
                        nc.vector.tensor_copy(bjT[j][:, c:c + 1], psb[:])


            # prelu chunk-pair engine schedule per (layer, chunk):
            # 'A' -> single ACT Prelu(bias) op; 'D' -> DVE add-bias +
            # GPSIMD scale + DVE max (prelu(h) = max(h, a*h), needs 0<=a<=1)
            if dve_ok:
                assign = {(0, 1): 'D', (2, 1): 'D'}
            else:
                assign = {}

            # ---- main loop: superpairs of 2 pixel-tile pairs ----
            # matmuls quad-weight-stationary: each (layer, chunk, k) weight
            # is loaded once per 4 px tiles (walrus dedupes the repeated
            # Ldweights); elementwise stays at pair granularity [128,1024].
            accL = None
            for sp in range(NT // 4):
                Ty_sb = ap.tile([128, 4, TP], bf, tag="tysb", name=f"ty{sp}")
                nc.sync.dma_start(Ty_sb[:], Ty_d[:, 4 * sp:4 * sp + 4, :])
                prev = [None, None]
                for j in range(L):
                    actj = [ap.tile([128, 2, 2 * TP], bf, tag=f"act{j}{h}",
                                    name=f"act{j}{h}_{sp}") for h in range(2)]
                    for c in range(2):
                        ps = [psm.tile([128, 2, TP], fp, tag="psmm",
                                       name=f"ps{j}{c}{h}_{sp}") for h in range(2)]
                        for k in range(2):
                            for q in range(4):
                                h, s_ = q // 2, q % 2
                                if j == 0:
                                    rhs = (Ty_sb[:, 2 * h + s_, :] if k == 0
                                           else Trep[:])
                                else:
                                    rhs = prev[h][:, k, TP * s_:TP * (s_ + 1)]
                                nc.tensor.matmul(
                                    ps[h][:, s_, :],
                                    wj[j][:, k, 128 * c:128 * (c + 1)], rhs,
                                    start=(k == 0), stop=(k == 1))
                        for h in range(2):
                            psf = ps[h].rearrange("p a b -> p (a b)")
                            if assign.get((j, c), 'A') == 'D':
                                h1 = dp.tile([128, 2 * TP], bf, tag="dveh",
                                             name=f"h{j}{c}{h}_{sp}")
                                t1 = dp.tile([128, 2 * TP], bf, tag="dvet",
                                             name=f"t{j}{c}{h}_{sp}")
                                nc.vector.tensor_scalar(
                                    h1[:], psf, bjT[j][:, c:c + 1], None, ADD)
                                nc.vector.tensor_scalar(
                                    t1[:], h1[:], alpha, None, MULT)
                                nc.vector.tensor_tensor(
                                    actj[h][:, c, :], h1[:], t1[:], MAX)
                            else:
                                nc.scalar.activation(
                                    actj[h][:, c, :], psf, PRELU,
                                    bias=bjT[j][:, c:c + 1], alpha=alpha)
                    prev = actj
                # last1: one PSUM bank per superpair (4 tiles, col groups)
                accL = psl.tile([128, TP], fp, tag="pslastb", name=f"accL{sp}")
                for k in range(2):
                    for q in range(4):
                        h, s_ = q // 2, q % 2
                        nc.tensor.matmul(
                            accL[32 * q:32 * q + C2, :], lwT[:, k, :],
                            prev[h][:, k, TP * s_:TP * (s_ + 1)],
                            start=(k == 0), stop=(k == 1),
                            tile_position=(0, 32 * q))
                souf = op.tile([128, TP], fp, tag="souf", name=f"souf{sp}")
                nc.scalar.activation(souf[0:99, :], accL[0:99, :],
                                     SILU, bias=lbrep[0:99, 0:1])
                for c in range(C2):
                    nc.sync.dma_start(out_r[c, 4 * sp:4 * sp + 4, :],
                                      souf[c:c + 97:32, :])
    return nc


def kernel(x, conv_w, conv_b, wfine_w, wfine_b, last1_w, last1_b, prelu_a,
           **_ignored):
    global _last_results
    from concourse.bass_utils import run_bass_kernel_spmd

    x = np.asarray(x)
    B = x.shape[0]
    assert x.shape == (B, C1, 16, 16) and B == 8, x.shape

    conv_w = np.asarray(conv_w, np.float32)      # [1028, 1024]
    conv_b = np.asarray(conv_b, np.float32)      # [1028]
    wfine_w = np.asarray(wfine_w, np.float32)    # [256, 256]
    wfine_b = np.asarray(wfine_b, np.float32)    # [256]
    last1_w = np.asarray(last1_w, np.float32)    # [3, 256]
    last1_b = np.asarray(last1_b, np.float32)    # [3]
    alpha = float(np.asarray(prelu_a).reshape(-1)[0])

    # host-side shared operands (bf16)
    cwT = np.ascontiguousarray(
        conv_w.T.reshape(8, 128, WD).transpose(1, 0, 2)).astype(bf16)
    cb = conv_b.reshape(1, WD).astype(bf16)
    wfT = np.ascontiguousarray(
        wfine_w.T.reshape(2, 128, NF).transpose(1, 0, 2)).astype(bf16)
    wfb = wfine_b.reshape(1, NF).astype(bf16)
    lwT = np.ascontiguousarray(
        last1_w.T.reshape(2, 128, C2).transpose(1, 0, 2)).astype(bf16)
    lbrep = np.zeros((128, 1), np.float32)
    for g in range(4):
        lbrep[32 * g:32 * g + C2, 0] = last1_b
    Tt = _host_tables()
    Trep = np.tile(Tt, (1, 4)).astype(bf16)
    Ty = np.ascontiguousarray(
        np.broadcast_to(Tt[:, :, None], (128, 128, 128)).reshape(128, NT, 512)
    ).astype(bf16)

    nc = _build_program(alpha)

    in_maps = []
    for b in range(B):
        xb = np.ascontiguousarray(
            x[b].reshape(8, 128, NF).transpose(1, 0, 2)).astype(bf16)
        in_maps.append({"xb": xb, "cwT": cwT, "cb": cb, "wfT": wfT,
                        "wfb": wfb, "lwT": lwT, "lbrep": lbrep,
                        "Trep": Trep, "Ty": Ty})

    res = run_bass_kernel_spmd(nc, in_maps, list(range(8)))
    _last_results = res
    out = np.stack([res.results[b]["out"].reshape(C2, IMG, IMG)
                    for b in range(B)])
    return out.astype(np.float32)


# ---------------------------------------------------------------------------
# Embedded walrus workaround (kernel.py must be self-contained): this walrus
# build accepts at most ONE sync wait per instruction; Tile attaches several.
# Split them into preceding single-wait NoOps at the BIR-JSON level, and make
# the TileContext tail drain emit one single-wait drain per logical proc.
# ---------------------------------------------------------------------------
import sys as _sys
import types as _types

_patch_mod = _types.ModuleType("bir_patch_embedded")
_patch_src = r'''
import json

def install():
    import concourse.bass_utils as _bu
    import concourse.bass2jax as _b2j
    import concourse.tile as _tile
    from concourse.vector_clock import ScopedClock, VectorClock

    if getattr(_bu, "_wait_legalizer_installed", False):
        return
    _bu._wait_legalizer_installed = True
    _orig_compile = _bu.compile_bir_kernel

    def _merge_ldweights(m):
        """Re-merge tile-legalize's split Ldweights into self-loading
        Matmults so walrus codegen can apply FWL / ldw dedupe."""
        for fn in m.get("functions", []):
            for bb in fn.get("blocks", []):
                instrs = bb.get("instructions", [])
                out = []
                i = 0
                while i < len(instrs):
                    ins = instrs[i]
                    if ins.get("opcode") == "Ldweights":
                        wap = json.dumps(ins["ins"][0], sort_keys=True)
                        # find the next Matmult on this engine using these
                        # weights (stationary operand = ins[1])
                        tgt = None
                        for k in range(i + 1, min(i + 8, len(instrs))):
                            nxt = instrs[k]
                            if nxt.get("engine") != ins.get("engine"):
                                continue
                            if nxt.get("opcode") == "Matmult" and json.dumps(
                                    nxt["ins"][1], sort_keys=True) == wap:
                                tgt = nxt
                            break
                        if tgt is not None:
                            tgt["ldweights"] = True
                            si, ti = ins.get("sync_info") or {}, tgt.setdefault(
                                "sync_info", {"on_wait": [], "on_update": []})
                            ti.setdefault("on_wait", []).extend(
                                si.get("on_wait") or [])
                            ti.setdefault("on_update", []).extend(
                                si.get("on_update") or [])
                            i += 1
                            continue
                    out.append(ins)
                    i += 1
                bb["instructions"] = out
        return m

    def _legalize_waits(bir_json):
        m = json.loads(bir_json)
        m = _merge_ldweights(m)
        cnt = 0
        changed = True
        for fn in m.get("functions", []):
            for bb in fn.get("blocks", []):
                new_instrs = []
                for ins in bb.get("instructions", []):
                    si = ins.get("sync_info")
                    ow = (si or {}).get("on_wait") or []
                    if len(ow) > 1:
                        changed = True
                        for w in ow[:-1]:
                            cnt += 1
                            new_instrs.append({
                                "engine": ins["engine"],
                                "ins": [], "outs": [],
                                "name": "WSPLIT-%d" % cnt,
                                "opcode": "NoOp",
                                "sync_info": {"on_update": [], "on_wait": [w]},
                                "debug": ins.get("debug", 0),
                            })
                        si["on_wait"] = [ow[-1]]
                    new_instrs.append(ins)
                bb["instructions"] = new_instrs
        if not changed:
            return bir_json
        return json.dumps(m).encode()

    def _compile_legalized(bir_json, tmpdir, neff_name="file.neff"):
        return _orig_compile(_legalize_waits(bir_json), tmpdir, neff_name)

    _bu.compile_bir_kernel = _compile_legalized
    _b2j.compile_bir_kernel = _compile_legalized

    import os
    if os.environ.get("BASS_LDW_OPT", "1") != "0":
        _orig_verify = _bu.bir_verify_and_optimise

        def _verify_ldwopt(tmpdir, inp="bir.json", outp="file.neff", arch=None,
                           *, dve_root=None):
            saved = _bu.run_command

            def run_cmd(cmd, **kw):
                cmd = [c.replace("--enable-ldw-opt=false",
                                 "--enable-ldw-opt=true")
                       if isinstance(c, str) else c for c in cmd]
                return saved(cmd, **kw)
            _bu.run_command = run_cmd
            try:
                return _orig_verify(tmpdir, inp, outp, arch, dve_root=dve_root)
            finally:
                _bu.run_command = saved
        _bu.bir_verify_and_optimise = _verify_ldwopt

    def _drain_and_barrier_split(self, tick_clock, wait_clock):
        nc = self.nc
        vclock = tick_clock.global_clock
        n = len(vclock)
        for p in range(n):
            t = vclock[p]
            if t <= 0:
                continue
            v = VectorClock([0] * n)
            v.require_at_least(p, t)
            d = nc.sync.drain()
            wait_clock.add_sem_waits(d.ins, ScopedClock({None: v}))
        nc.all_engine_barrier()
        popped = nc._tile_sem_poison_stack.pop()
        assert popped is self._sem_poison
        nc.clear_and_free_semaphores(list(self.sems.allocated().values()))
        nc.all_engine_barrier()

    _tile.TileContext._drain_and_barrier = _drain_and_barrier_split
'''
exec(_patch_src, _patch_mod.__dict__)
_sys.modules["bir_patch_embedded"] = _patch_mod
